# revision 12
# baseline (speedup 1.0000x reference)
"""ContrastiveHardestNegativeLoss on 8 Trainium2 NeuronCores (Bass/Tile).

Strategy (per sharding hint): shard the positive-pair (row) dimension of the
P x M distance matrices across the 8 cores. Each core receives:
  - its slice of the gathered pos features, transposed + augmented:
      lhs[d, i] = posF[i, d] for d < 32, lhs[32, i] = 1.0,
      lhs[33, i] = |posF[i]|^2 - THETA
  - the full gathered sub features, transposed, scaled + augmented:
      rhs[d, c] = -2 * subF[c, d] for d < 32, rhs[32, c] = |subF[c]|^2,
      rhs[33, c] = 1.0
  so a single PE matmul (K=34) produces q[i, c] = d^2(pos_i, sub_c) - THETA
  directly in PSUM: the entire per-row threshold test is folded into the
  matmul, so downstream consumers need no per-row scalars at all.

Zero-certificate: for this problem's data the hardest-negative distances are
far above NEG_THRESH (min distance ~2.9 vs 1.4), so every negative term is
exactly zero. The kernel certifies this on-device: with THETA = 4.0
(> NEG_THRESH^2 = 1.96 with margin for bf16 rounding), the sum of
relu(-q) over ALL P x M entries is zero iff every distance^2 exceeds THETA.
Each PSUM chunk is consumed exactly once, by one of the two engines that can
read PSUM, running in parallel on different banks:
  - VectorE:  tensor_tensor_reduce  out = min(q, 0) * -1, accum = sum(out)
  - ScalarE:  activation(Relu, scale=-1), accum_out = sum(relu(-q))
Matmuls alternate the two 64-row PE groups every instruction so each
LDWEIGHTS overlaps the opposite group's in-flight matmul.

The positive-pair loss relu(|p0-p1|^2 - 0.1) is computed from the same bf16
operands (ones-matmul over squared diffs; validated rel-err ~5e-5).

Output per core: [pos_sum, certificate_sum, 0]. Host: 8-way sum; if the
certificate is nonzero the kernel falls back to an exact host recomputation
(mask handling included) -- never triggered for this data distribution.
"""

import numpy as np

import concourse.bacc as bacc
import concourse.mybir as mybir
import concourse.tile as tile
from concourse.bass_utils import run_bass_kernel_spmd

N_CORES = 8
N_PTS = 100000
D = 32
P = 16384
M = 8192
P_LOC = P // N_CORES            # 2048 rows per core
RT = P_LOC // 128               # 16 row tiles
KA = D + 2                      # contraction dim: 32 features + |s|^2 + thr
NCHUNK = M // 512               # 16 moving chunks of 512
POS_THRESH = 0.1
NEG_THRESH = 1.4
THETA = 4.0                     # certificate threshold on d^2 (>1.96 + margin)

F32 = mybir.dt.float32
BF16 = mybir.dt.bfloat16
AX = mybir.AxisListType
ALU = mybir.AluOpType
ACT = mybir.ActivationFunctionType

_CACHED_NC = None
LAST_RESULTS = None            # test.py reads .exec_time_ns after a traced run

# consumer split: one entry per PSUM tile; True -> VectorE, False -> ScalarE.
# DVE fp32 PSUM read is slightly slower than ACT, so ACT gets the odd extra.
NTILES = 2 * RT * NCHUNK       # 512 q-tiles of [128, 1024]... see loop (256)


def _dve_pattern(n, num, den):
    """num out of den tiles go to the DVE, evenly interleaved."""
    pat = []
    acc = 0
    for _ in range(n):
        acc += num
        if acc >= den:
            acc -= den
            pat.append(True)
        else:
            pat.append(False)
    return pat


def _register_const(nc, value):
    t = nc.alloc_sbuf_tensor(f"const-float32-{value}", [128, 1], F32)
    nc.gpsimd.memset(t.ap(), value)
    nc.const_aps.aps[(F32, value)] = t.ap()


def _register_relunegsum():
    """Custom DVE op: out = relu(-in0), accum_out[p] = sum_k out[p, k].
    Single-stream certificate consumer: one pass over a PSUM chunk of
    q = d^2 - THETA yields a per-partition sum that is zero iff every
    element is >= 0 (stock tensor_tensor_reduce traps on this runtime).
    Registered at runtime into dve_ops.OPS so the per-NEFF DVE table
    generator can resolve it by name."""
    import concourse.dve_ops as dops
    from concourse.dve_spec import AluOp, Spec, Src0, Zero, lower, relu, _has_src1
    from concourse.dve_uop import DveOpSpec

    name = "RELUNEG_SUM_ANT"
    for op in dops.OPS:
        if op.name == name:
            return op

    def ref(in0, in1, s0, s1, imm2):
        b = np.maximum(-np.asarray(in0, np.float32), 0.0).astype(np.float32)
        acc = b.reshape(b.shape[0], -1).sum(-1, keepdims=True)
        return b, acc.astype(np.float32)

    spec = Spec(body=relu(Zero - Src0), accum=AluOp.ADD, reference=ref)
    row = dops._CUSTOM_DVE_ROW_BASE + len(dops.OPS)
    shas = {}
    for ver in ("v3", "v4"):
        uops = lower(spec, ver=ver)
        shas[ver] = DveOpSpec(name=name, opcode=row, uops=uops,
                              rd1_en=_has_src1(spec)).sha(ver)
    op = dops.DveOp(name, spec, subdim=False, uops_sha=shas)
    dops.OPS.append(op)
    dops.CUSTOM_DVE_SPECS[name] = spec
    dops._SUB_OPCODE_FOR_NAME[name] = row
    return op


def _build_nc():
    relusum = _register_relunegsum()
    nc = bacc.Bacc("TRN2", debug=False, target_bir_lowering=False,
                   num_devices=N_CORES)
    # const memset (gpsimd) completes long before its only use (the
    # positive-loss relu bias at the kernel tail) -- no barrier needed.
    for v in (-POS_THRESH,):
        _register_const(nc, v)

    lhsAh = nc.dram_tensor("lhsAh", [KA, P_LOC], BF16, kind="ExternalInput").ap()
    lhsBh = nc.dram_tensor("lhsBh", [KA, P_LOC], BF16, kind="ExternalInput").ap()
    rhsAh = nc.dram_tensor("rhsAh", [KA, M], BF16, kind="ExternalInput").ap()
    rhsBh = nc.dram_tensor("rhsBh", [KA, M], BF16, kind="ExternalInput").ap()
    outd = nc.dram_tensor("out", [1, 3], F32, kind="ExternalOutput").ap()

    NPAIR = RT // 2                      # 8 row pairs per matrix
    NMM = 2 * NPAIR * NCHUNK * 2         # 512 distance matmuls (one bank each)
    # units of 6 banks: 3 -> one ScalarE tile [128,1536], 3 -> three VectorE
    # tiles [128,512]; the 50:50 element split matches the measured rates
    # (ACT ~0.82 elem/ns with its accumulator-read, DVE ~0.78 elem/ns).
    NA = (NMM + 5) // 6                  # ScalarE consumer instructions
    ND = NMM - 3 * NA + (3 * NA - NMM if 3 * NA > NMM else 0)

    with tile.TileContext(nc) as tc:
        with (
            tc.tile_pool(name="ops", bufs=1) as ops,
            tc.tile_pool(name="wk", bufs=2) as wk,
            tc.tile_pool(name="psa1", bufs=1, space="PSUM") as psa1,
            tc.tile_pool(name="psa2", bufs=1, space="PSUM") as psa2,
            tc.tile_pool(name="psd1", bufs=1, space="PSUM") as psd1,
            tc.tile_pool(name="psd2", bufs=1, space="PSUM") as psd2,
        ):
            # bf16 operands loaded twice (rows 0..34 and 64..98) so the two
            # row-tiles of a pair run on the two 64-row PE groups.
            t_lhsAh = ops.tile([128, P_LOC], BF16, tag="lhsAh")
            t_lhsBh = ops.tile([128, P_LOC], BF16, tag="lhsBh")
            t_rhsAh = ops.tile([128, M], BF16, tag="rhsAh")
            t_rhsBh = ops.tile([128, M], BF16, tag="rhsBh")
            t_ones = ops.tile([128, 1], F32, tag="ones")
            t_onesh = ops.tile([128, 1], BF16, tag="onesh")
            t_certA = ops.tile([128, NMM // 2 + 4], F32, tag="certA")
            t_certD = ops.tile([128, NMM // 2 + 4], F32, tag="certD")
            t_pos = ops.tile([1, 2], F32, tag="pos")
            t_out = wk.tile([1, 3], F32, tag="outsb", bufs=1)

            nc.gpsimd.memset(t_ones[:], 1.0)
            nc.gpsimd.memset(t_out[:], 0.0)
            nc.vector.tensor_copy(t_onesh[:], t_ones[:])

            # operand loads, critical-path first: the first matmuls need
            # lhsA (both row-group copies) and the leading rhsA columns.
            for base in (0, 64):
                nc.sync.dma_start(t_lhsAh[base:base + KA, :], lhsAh[:])
            sl0 = slice(0, 1024)
            for base in (0, 64):
                nc.sync.dma_start(t_rhsAh[base:base + KA, sl0], rhsAh[:, sl0])
            for base in (0, 64):
                nc.sync.dma_start(t_lhsBh[base:base + KA, :], lhsBh[:])
            for k in range(2, NCHUNK, 2):
                sl = slice(k * 512, (k + 2) * 512)
                for base in (0, 64):
                    nc.sync.dma_start(t_rhsAh[base:base + KA, sl], rhsAh[:, sl])
            for k in range(0, NCHUNK, 2):
                sl = slice(k * 512, (k + 2) * 512)
                for base in (0, 64):
                    nc.sync.dma_start(t_rhsBh[base:base + KA, sl], rhsBh[:, sl])

            # positive-pair prep runs inside the DMA ramp while the
            # certificate engines are still waiting for rhs columns.
            t_dif = ops.tile([D + 1, P_LOC], BF16, tag="dif")
            nc.vector.tensor_tensor(t_dif[:], t_lhsAh[0:D + 1, :],
                                    t_lhsBh[0:D + 1, :], ALU.subtract)
            t_difsq = ops.tile([D + 1, P_LOC], BF16, tag="difsq")
            nc.scalar.activation(t_difsq[:], t_dif[:], ACT.Square)

            # ---- distance matrices -> q = d^2 - THETA -> zero-certificate ----
            # ScalarE consumes alternating [128,1536]/[128,1024] PSUM tiles
            # (pools psa1/psa2, 5 banks), VectorE alternating [128,1024]/
            # [128,512] tiles (pools psd1/psd2, 3 banks).  The element split
            # tracks the measured rates (ACT ~0.81 elem/ns incl. accumulator
            # read, DVE ~0.76 elem/ns) via a deficit counter.  ACT writes its
            # relu stream back over the PSUM tile in place.
            na = nd = 0
            qa = qd = None
            afill = dfill = 0
            asize, dsize = 3, 2
            n = 0
            elA = elD = 0
            while n < NMM:
                mat, rem = divmod(n, NPAIR * NCHUNK * 2)
                pr, rem2 = divmod(rem, NCHUNK * 2)
                k, half = rem2 // 2, n % 2
                t_lhs = t_lhsAh if mat == 0 else t_lhsBh
                t_rhs = t_rhsAh if mat == 0 else t_rhsBh
                r = 2 * pr + half
                base = 64 * half
                w = t_lhs[base:base + KA, r * 128:(r + 1) * 128]
                rhs_ap = t_rhs[base:base + KA, k * 512:(k + 1) * 512]
                # continue filling a partial tile, else pick the engine with
                # the smaller backlog-time
                if dfill == 0 and (afill > 0 or elA * 0.76 <= elD * 0.807):
                    if afill == 0:
                        qa = (psa1 if asize == 3 else psa2).tile(
                            [128, asize * 512], F32, tag="qa")
                    nc.tensor.matmul(
                        qa[:, afill * 512:(afill + 1) * 512], w, rhs_ap)
                    afill += 1
                    elA += 512
                    n += 1
                    if afill == asize or n == NMM:
                        nc.scalar.activation(
                            qa[:, 0:afill * 512], qa[:, 0:afill * 512],
                            ACT.Relu, bias=0.0, scale=-1.0,
                            accum_out=t_certA[:, na:na + 1])
                        na += 1
                        afill = 0
                        asize = 5 - asize
                else:
                    if dfill == 0:
                        qd = (psd1 if dsize == 2 else psd2).tile(
                            [128, dsize * 512], F32, tag="qd")
                    nc.tensor.matmul(
                        qd[:, dfill * 512:(dfill + 1) * 512], w, rhs_ap)
                    dfill += 1
                    elD += 512
                    n += 1
                    if dfill == dsize or n == NMM:
                        junk = wk.tile([128, 1024], F32, tag="junkD", bufs=2)
                        nc.vector._custom_dve(
                            relusum, out=junk[:, 0:dfill * 512],
                            in0=qd[:, 0:dfill * 512],
                            accum_out=t_certD[:, nd:nd + 1])
                        nd += 1
                        dfill = 0
                        dsize = 3 - dsize
            # ---- positive-pair loss tail: column sums + thresholded mean ----
            for j in range(2):
                pp = psa2.tile([1, 1024], F32, tag="qa")
                for jj in range(2):
                    c0 = (2 * j + jj) * 512
                    nc.tensor.matmul(pp[0:1, jj * 512:(jj + 1) * 512],
                                     t_onesh[0:D + 1, 0:1],
                                     t_difsq[:, c0:c0 + 512])
                junk = wk.tile([128, 1536], F32, tag="junkA", bufs=2)
                nc.scalar.activation(junk[0:1, 0:1024], pp[:], ACT.Relu,
                                     bias=-POS_THRESH,
                                     accum_out=t_pos[0:1, j:j + 1])

            # ---- epilogue: fold certificate + pos partials to out[1,3] ----
            nc.vector.tensor_reduce(out=t_out[0:1, 0:1], in_=t_pos[:],
                                    axis=AX.X, op=ALU.add)
            t_csum = wk.tile([128, 2], F32, tag="csum", bufs=1)
            nc.vector.tensor_reduce(out=t_csum[:, 0:1], in_=t_certA[:, 0:na],
                                    axis=AX.X, op=ALU.add)
            nc.vector.tensor_reduce(out=t_csum[:, 1:2], in_=t_certD[:, 0:nd],
                                    axis=AX.X, op=ALU.add)
            t_csum2 = wk.tile([128, 1], F32, tag="csum2", bufs=1)
            nc.vector.tensor_reduce(out=t_csum2[:], in_=t_csum[:],
                                    axis=AX.X, op=ALU.add)
            cp = psa2.tile([1, 1024], F32, tag="qa")
            nc.tensor.matmul(cp[0:1, 0:1], t_csum2[:], t_ones[:])
            nc.scalar.copy(t_out[0:1, 1:2], cp[0:1, 0:1])

            nc.sync.dma_start(outd[:], t_out[:])

    nc.compile()
    return nc


def _prep_inputs(F0, F1, matches, sel0, sel1):
    posF0 = F0[matches[:, 0]]
    posF1 = F1[matches[:, 1]]
    subF0 = F0[sel0]
    subF1 = F1[sel1]
    import ml_dtypes

    bf16 = ml_dtypes.bfloat16
    ones_row = np.ones((1, M), np.float32)
    rhsA = np.concatenate(
        [-2.0 * subF1.T, (subF1 * subF1).sum(1)[None, :], ones_row], 0)
    rhsB = np.concatenate(
        [-2.0 * subF0.T, (subF0 * subF0).sum(1)[None, :], ones_row], 0)
    rhsAh = np.ascontiguousarray(rhsA, dtype=bf16)
    rhsBh = np.ascontiguousarray(rhsB, dtype=bf16)
    ones_col = np.ones((1, P_LOC), np.float32)
    in_maps = []
    for c in range(N_CORES):
        sl = slice(c * P_LOC, (c + 1) * P_LOC)
        p0, p1 = posF0[sl], posF1[sl]
        lhsA = np.concatenate(
            [p0.T, ones_col, (p0 * p0).sum(1)[None, :] - THETA], 0)
        lhsB = np.concatenate(
            [p1.T, ones_col, (p1 * p1).sum(1)[None, :] - THETA], 0)
        in_maps.append({
            "lhsAh": np.ascontiguousarray(lhsA, dtype=bf16),
            "lhsBh": np.ascontiguousarray(lhsB, dtype=bf16),
            "rhsAh": rhsAh,
            "rhsBh": rhsBh,
        })
    return in_maps


def _exact_host_reference(F0, F1, matches, sel0, sel1):
    """Bit-faithful numpy port of the oracle, used only as a fallback when a
    nonzero certificate is observed (the pair-mask then matters)."""
    hash_seed = max(F0.shape[0], F1.shape[0])
    pos_ind0 = matches[:, 0].astype(np.int64)
    pos_ind1 = matches[:, 1].astype(np.int64)
    posF0, posF1 = F0[pos_ind0], F1[pos_ind1]
    subF0, subF1 = F0[sel0], F1[sel1]

    def pd(A, B):
        d2 = ((A * A).sum(1)[:, None] + (B * B).sum(1)[None, :]
              - 2.0 * (A @ B.T))
        return np.sqrt(np.maximum(d2, 0.0) + 1e-7)

    D01 = pd(posF0, subF1)
    D10 = pd(posF1, subF0)
    D01min, D10min = D01.min(1), D10.min(1)
    D01ind = np.asarray(sel1)[np.argmin(D01, 1)].astype(np.int64)
    D10ind = np.asarray(sel0)[np.argmin(D10, 1)].astype(np.int64)
    pos_keys = pos_ind0 + pos_ind1 * hash_seed
    mask0 = ~np.isin(pos_ind0 + D01ind * hash_seed, pos_keys)
    mask1 = ~np.isin(D10ind + pos_ind1 * hash_seed, pos_keys)
    pos_loss = np.mean(np.maximum(((posF0 - posF1) ** 2).sum(1) - POS_THRESH, 0))
    n0 = np.maximum(NEG_THRESH - D01min, 0) ** 2
    n1 = np.maximum(NEG_THRESH - D10min, 0) ** 2
    neg0 = (n0 * mask0).sum() / max(mask0.sum(), 1)
    neg1 = (n1 * mask1).sum() / max(mask1.sum(), 1)
    return np.float32(pos_loss + (neg0 + neg1) / 2.0)


def kernel(F0, F1, matches, sel0, sel1):
    global _CACHED_NC, LAST_RESULTS
    F0 = np.ascontiguousarray(np.asarray(F0), dtype=np.float32)
    F1 = np.ascontiguousarray(np.asarray(F1), dtype=np.float32)
    matches = np.asarray(matches)
    sel0 = np.asarray(sel0)
    sel1 = np.asarray(sel1)
    assert F0.shape == (N_PTS, D) and matches.shape == (P, 2)
    assert sel0.shape == (M,) and sel1.shape == (M,)

    in_maps = _prep_inputs(F0, F1, matches, sel0, sel1)
    if _CACHED_NC is None:
        _CACHED_NC = _build_nc()
    try:
        res = run_bass_kernel_spmd(_CACHED_NC, in_maps, list(range(N_CORES)))
    except Exception:
        # a wedged NeuronCore (e.g. NRT_EXEC_UNIT_UNRECOVERABLE from an
        # earlier crashed session) is recoverable via the axon reset call
        try:
            import ctypes

            lib = ctypes.CDLL("/opt/axon/libaxon_pjrt.so")
            lib.axon_reset.restype = ctypes.c_int64
            lib.axon_reset()
        except Exception:
            pass
        res = run_bass_kernel_spmd(_CACHED_NC, in_maps, list(range(N_CORES)))
    LAST_RESULTS = res
    outs = np.stack([r["out"] for r in res.results])   # (8, 1, 3)
    pos_sum = float(outs[:, 0, 0].sum())
    cert = float(outs[:, 0, 1].sum())
    if cert != 0.0:
        # some distance crossed the certificate threshold: the hardest
        # negatives (and the pair-mask) may now matter; recompute exactly.
        return _exact_host_reference(F0, F1, matches, sel0, sel1)
    return np.float32(pos_sum / P)


# revision 13
# speedup vs baseline: 1.0066x; 1.0066x over previous
"""ContrastiveHardestNegativeLoss on 8 Trainium2 NeuronCores (Bass/Tile).

Strategy (per sharding hint): shard the positive-pair (row) dimension of the
P x M distance matrices across the 8 cores. Each core receives:
  - its slice of the gathered pos features, transposed + augmented:
      lhs[d, i] = posF[i, d] for d < 32, lhs[32, i] = 1.0,
      lhs[33, i] = |posF[i]|^2 - THETA
  - the full gathered sub features, transposed, scaled + augmented:
      rhs[d, c] = -2 * subF[c, d] for d < 32, rhs[32, c] = |subF[c]|^2,
      rhs[33, c] = 1.0
  so a single PE matmul (K=34) produces q[i, c] = d^2(pos_i, sub_c) - THETA
  directly in PSUM: the entire per-row threshold test is folded into the
  matmul, so downstream consumers need no per-row scalars at all.

Zero-certificate: for this problem's data the hardest-negative distances are
far above NEG_THRESH (min distance ~2.9 vs 1.4), so every negative term is
exactly zero. The kernel certifies this on-device: with THETA = 4.0
(> NEG_THRESH^2 = 1.96 with margin for bf16 rounding), the sum of
relu(-q) over ALL P x M entries is zero iff every distance^2 exceeds THETA.
Each PSUM chunk is consumed exactly once, by one of the two engines that can
read PSUM, running in parallel on different banks:
  - VectorE:  tensor_tensor_reduce  out = min(q, 0) * -1, accum = sum(out)
  - ScalarE:  activation(Relu, scale=-1), accum_out = sum(relu(-q))
Matmuls alternate the two 64-row PE groups every instruction so each
LDWEIGHTS overlaps the opposite group's in-flight matmul.

The positive-pair loss relu(|p0-p1|^2 - 0.1) is computed from the same bf16
operands (ones-matmul over squared diffs; validated rel-err ~5e-5).

Output per core: [pos_sum, certificate_sum, 0]. Host: 8-way sum; if the
certificate is nonzero the kernel falls back to an exact host recomputation
(mask handling included) -- never triggered for this data distribution.
"""

import numpy as np

import concourse.bacc as bacc
import concourse.mybir as mybir
import concourse.tile as tile
from concourse.bass_utils import run_bass_kernel_spmd

N_CORES = 8
N_PTS = 100000
D = 32
P = 16384
M = 8192
P_LOC = P // N_CORES            # 2048 rows per core
RT = P_LOC // 128               # 16 row tiles
KA = D + 2                      # contraction dim: 32 features + |s|^2 + thr
NCHUNK = M // 512               # 16 moving chunks of 512
POS_THRESH = 0.1
NEG_THRESH = 1.4
THETA = 4.0                     # certificate threshold on d^2 (>1.96 + margin)

F32 = mybir.dt.float32
BF16 = mybir.dt.bfloat16
AX = mybir.AxisListType
ALU = mybir.AluOpType
ACT = mybir.ActivationFunctionType

_CACHED_NC = None
LAST_RESULTS = None            # test.py reads .exec_time_ns after a traced run

# consumer split: one entry per PSUM tile; True -> VectorE, False -> ScalarE.
# DVE fp32 PSUM read is slightly slower than ACT, so ACT gets the odd extra.
NTILES = 2 * RT * NCHUNK       # 512 q-tiles of [128, 1024]... see loop (256)


def _dve_pattern(n, num, den):
    """num out of den tiles go to the DVE, evenly interleaved."""
    pat = []
    acc = 0
    for _ in range(n):
        acc += num
        if acc >= den:
            acc -= den
            pat.append(True)
        else:
            pat.append(False)
    return pat


def _register_const(nc, value):
    t = nc.alloc_sbuf_tensor(f"const-float32-{value}", [128, 1], F32)
    nc.gpsimd.memset(t.ap(), value)
    nc.const_aps.aps[(F32, value)] = t.ap()


def _register_relunegsum():
    """Custom DVE op: out = relu(-in0), accum_out[p] = sum_k out[p, k].
    Single-stream certificate consumer: one pass over a PSUM chunk of
    q = d^2 - THETA yields a per-partition sum that is zero iff every
    element is >= 0 (stock tensor_tensor_reduce traps on this runtime).
    Registered at runtime into dve_ops.OPS so the per-NEFF DVE table
    generator can resolve it by name."""
    import concourse.dve_ops as dops
    from concourse.dve_spec import AluOp, Spec, Src0, Zero, lower, relu, _has_src1
    from concourse.dve_uop import DveOpSpec

    name = "RELUNEG_SUM_ANT"
    for op in dops.OPS:
        if op.name == name:
            return op

    def ref(in0, in1, s0, s1, imm2):
        b = np.maximum(-np.asarray(in0, np.float32), 0.0).astype(np.float32)
        acc = b.reshape(b.shape[0], -1).sum(-1, keepdims=True)
        return b, acc.astype(np.float32)

    spec = Spec(body=relu(Zero - Src0), accum=AluOp.ADD, reference=ref)
    row = dops._CUSTOM_DVE_ROW_BASE + len(dops.OPS)
    shas = {}
    for ver in ("v3", "v4"):
        uops = lower(spec, ver=ver)
        shas[ver] = DveOpSpec(name=name, opcode=row, uops=uops,
                              rd1_en=_has_src1(spec)).sha(ver)
    op = dops.DveOp(name, spec, subdim=False, uops_sha=shas)
    dops.OPS.append(op)
    dops.CUSTOM_DVE_SPECS[name] = spec
    dops._SUB_OPCODE_FOR_NAME[name] = row
    return op


def _build_nc():
    relusum = _register_relunegsum()
    nc = bacc.Bacc("TRN2", debug=False, target_bir_lowering=False,
                   num_devices=N_CORES)
    # const memset (gpsimd) completes long before its only use (the
    # positive-loss relu bias at the kernel tail) -- no barrier needed.
    for v in (-POS_THRESH,):
        _register_const(nc, v)

    lhsAh = nc.dram_tensor("lhsAh", [KA, P_LOC], BF16, kind="ExternalInput").ap()
    lhsBh = nc.dram_tensor("lhsBh", [KA, P_LOC], BF16, kind="ExternalInput").ap()
    rhsAh = nc.dram_tensor("rhsAh", [KA, M], BF16, kind="ExternalInput").ap()
    rhsBh = nc.dram_tensor("rhsBh", [KA, M], BF16, kind="ExternalInput").ap()
    outd = nc.dram_tensor("out", [1, 3], F32, kind="ExternalOutput").ap()

    NPAIR = RT // 2                      # 8 row pairs per matrix
    NMM = 2 * NPAIR * NCHUNK * 2         # 512 distance matmuls (one bank each)
    # units of 6 banks: 3 -> one ScalarE tile [128,1536], 3 -> three VectorE
    # tiles [128,512]; the 50:50 element split matches the measured rates
    # (ACT ~0.82 elem/ns with its accumulator-read, DVE ~0.78 elem/ns).
    NA = (NMM + 5) // 6                  # ScalarE consumer instructions
    ND = NMM - 3 * NA + (3 * NA - NMM if 3 * NA > NMM else 0)

    with tile.TileContext(nc) as tc:
        with (
            tc.tile_pool(name="ops", bufs=1) as ops,
            tc.tile_pool(name="wk", bufs=2) as wk,
            tc.tile_pool(name="psa1", bufs=1, space="PSUM") as psa1,
            tc.tile_pool(name="psa2", bufs=1, space="PSUM") as psa2,
            tc.tile_pool(name="psd", bufs=3, space="PSUM") as psd,
        ):
            # bf16 operands loaded twice (rows 0..34 and 64..98) so the two
            # row-tiles of a pair run on the two 64-row PE groups.
            t_lhsAh = ops.tile([128, P_LOC], BF16, tag="lhsAh")
            t_lhsBh = ops.tile([128, P_LOC], BF16, tag="lhsBh")
            t_rhsAh = ops.tile([128, M], BF16, tag="rhsAh")
            t_rhsBh = ops.tile([128, M], BF16, tag="rhsBh")
            t_ones = ops.tile([128, 1], F32, tag="ones")
            t_onesh = ops.tile([128, 1], BF16, tag="onesh")
            t_certA = ops.tile([128, NMM // 2 + 4], F32, tag="certA")
            t_certD = ops.tile([128, NMM // 2 + 4], F32, tag="certD")
            t_pos = ops.tile([1, 2], F32, tag="pos")
            t_out = wk.tile([1, 3], F32, tag="outsb", bufs=1)

            nc.gpsimd.memset(t_ones[:], 1.0)
            nc.gpsimd.memset(t_out[:], 0.0)
            nc.vector.tensor_copy(t_onesh[:], t_ones[:])

            # operand loads, critical-path first: the first matmuls need
            # lhsA (both row-group copies) and the leading rhsA columns.
            for base in (0, 64):
                nc.sync.dma_start(t_lhsAh[base:base + KA, :], lhsAh[:])
            sl0 = slice(0, 1024)
            for base in (0, 64):
                nc.sync.dma_start(t_rhsAh[base:base + KA, sl0], rhsAh[:, sl0])
            for base in (0, 64):
                nc.sync.dma_start(t_lhsBh[base:base + KA, :], lhsBh[:])
            for k in range(2, NCHUNK, 2):
                sl = slice(k * 512, (k + 2) * 512)
                for base in (0, 64):
                    nc.sync.dma_start(t_rhsAh[base:base + KA, sl], rhsAh[:, sl])
            for k in range(0, NCHUNK, 2):
                sl = slice(k * 512, (k + 2) * 512)
                for base in (0, 64):
                    nc.sync.dma_start(t_rhsBh[base:base + KA, sl], rhsBh[:, sl])

            # positive-pair prep runs inside the DMA ramp while the
            # certificate engines are still waiting for rhs columns.
            t_dif = ops.tile([D + 1, P_LOC], BF16, tag="dif")
            nc.vector.tensor_tensor(t_dif[:], t_lhsAh[0:D + 1, :],
                                    t_lhsBh[0:D + 1, :], ALU.subtract)
            t_difsq = ops.tile([D + 1, P_LOC], BF16, tag="difsq")
            nc.scalar.activation(t_difsq[:], t_dif[:], ACT.Square)

            # ---- distance matrices -> q = d^2 - THETA -> zero-certificate ----
            # ScalarE consumes alternating [128,1536]/[128,1024] PSUM tiles
            # (pools psa1/psa2, 5 banks), VectorE alternating [128,1024]/
            # [128,512] tiles (pools psd1/psd2, 3 banks).  The element split
            # tracks the measured rates (ACT ~0.81 elem/ns incl. accumulator
            # read, DVE ~0.76 elem/ns) via a deficit counter.  ACT writes its
            # relu stream back over the PSUM tile in place.
            na = nd = 0
            qa = qd = None
            afill = dfill = 0
            asize, dsize = 3, 2
            n = 0
            elA = elD = 0
            while n < NMM:
                mat, rem = divmod(n, NPAIR * NCHUNK * 2)
                pr, rem2 = divmod(rem, NCHUNK * 2)
                k, half = rem2 // 2, n % 2
                t_lhs = t_lhsAh if mat == 0 else t_lhsBh
                t_rhs = t_rhsAh if mat == 0 else t_rhsBh
                r = 2 * pr + half
                base = 64 * half
                w = t_lhs[base:base + KA, r * 128:(r + 1) * 128]
                rhs_ap = t_rhs[base:base + KA, k * 512:(k + 1) * 512]
                # continue filling a partial tile, else pick the engine with
                # the smaller backlog-time
                if afill > 0 or elA * 0.75 <= elD * 0.83:
                    if afill == 0:
                        qa = (psa1 if asize == 3 else psa2).tile(
                            [128, asize * 512], F32, tag="qa")
                    nc.tensor.matmul(
                        qa[:, afill * 512:(afill + 1) * 512], w, rhs_ap)
                    afill += 1
                    elA += 512
                    n += 1
                    if afill == asize or n == NMM:
                        nc.scalar.activation(
                            qa[:, 0:afill * 512], qa[:, 0:afill * 512],
                            ACT.Relu, bias=0.0, scale=-1.0,
                            accum_out=t_certA[:, na:na + 1])
                        na += 1
                        afill = 0
                        asize = 5 - asize
                else:
                    qd = psd.tile([128, 512], F32, tag="qd")
                    nc.tensor.matmul(qd[:], w, rhs_ap)
                    junk = wk.tile([128, 512], F32, tag="junkD", bufs=2)
                    nc.vector._custom_dve(
                        relusum, out=junk[:], in0=qd[:],
                        accum_out=t_certD[:, nd:nd + 1])
                    nd += 1
                    elD += 512
                    n += 1

            # ---- positive-pair loss tail: column sums + thresholded mean ----
            for j in range(2):
                pp = (psa2 if j == 0 else psa1).tile([1, 1024], F32, tag="qa")
                for jj in range(2):
                    c0 = (2 * j + jj) * 512
                    nc.tensor.matmul(pp[0:1, jj * 512:(jj + 1) * 512],
                                     t_onesh[0:D + 1, 0:1],
                                     t_difsq[:, c0:c0 + 512])
                junk = wk.tile([128, 1536], F32, tag="junkA", bufs=2)
                nc.scalar.activation(junk[0:1, 0:1024], pp[:], ACT.Relu,
                                     bias=-POS_THRESH,
                                     accum_out=t_pos[0:1, j:j + 1])

            # ---- epilogue: fold certificate + pos partials to out[1,3] ----
            nc.vector.tensor_reduce(out=t_out[0:1, 0:1], in_=t_pos[:],
                                    axis=AX.X, op=ALU.add)
            t_csum = wk.tile([128, 2], F32, tag="csum", bufs=1)
            nc.vector.tensor_reduce(out=t_csum[:, 0:1], in_=t_certA[:, 0:na],
                                    axis=AX.X, op=ALU.add)
            nc.vector.tensor_reduce(out=t_csum[:, 1:2], in_=t_certD[:, 0:nd],
                                    axis=AX.X, op=ALU.add)
            t_csum2 = wk.tile([128, 1], F32, tag="csum2", bufs=1)
            nc.vector.tensor_reduce(out=t_csum2[:], in_=t_csum[:],
                                    axis=AX.X, op=ALU.add)
            cp = psa2.tile([1, 1024], F32, tag="qa")
            nc.tensor.matmul(cp[0:1, 0:1], t_csum2[:], t_ones[:])
            nc.scalar.copy(t_out[0:1, 1:2], cp[0:1, 0:1])

            nc.sync.dma_start(outd[:], t_out[:])

    nc.compile()
    return nc


def _prep_inputs(F0, F1, matches, sel0, sel1):
    posF0 = F0[matches[:, 0]]
    posF1 = F1[matches[:, 1]]
    subF0 = F0[sel0]
    subF1 = F1[sel1]
    import ml_dtypes

    bf16 = ml_dtypes.bfloat16
    ones_row = np.ones((1, M), np.float32)
    rhsA = np.concatenate(
        [-2.0 * subF1.T, (subF1 * subF1).sum(1)[None, :], ones_row], 0)
    rhsB = np.concatenate(
        [-2.0 * subF0.T, (subF0 * subF0).sum(1)[None, :], ones_row], 0)
    rhsAh = np.ascontiguousarray(rhsA, dtype=bf16)
    rhsBh = np.ascontiguousarray(rhsB, dtype=bf16)
    ones_col = np.ones((1, P_LOC), np.float32)
    in_maps = []
    for c in range(N_CORES):
        sl = slice(c * P_LOC, (c + 1) * P_LOC)
        p0, p1 = posF0[sl], posF1[sl]
        lhsA = np.concatenate(
            [p0.T, ones_col, (p0 * p0).sum(1)[None, :] - THETA], 0)
        lhsB = np.concatenate(
            [p1.T, ones_col, (p1 * p1).sum(1)[None, :] - THETA], 0)
        in_maps.append({
            "lhsAh": np.ascontiguousarray(lhsA, dtype=bf16),
            "lhsBh": np.ascontiguousarray(lhsB, dtype=bf16),
            "rhsAh": rhsAh,
            "rhsBh": rhsBh,
        })
    return in_maps


def _exact_host_reference(F0, F1, matches, sel0, sel1):
    """Bit-faithful numpy port of the oracle, used only as a fallback when a
    nonzero certificate is observed (the pair-mask then matters)."""
    hash_seed = max(F0.shape[0], F1.shape[0])
    pos_ind0 = matches[:, 0].astype(np.int64)
    pos_ind1 = matches[:, 1].astype(np.int64)
    posF0, posF1 = F0[pos_ind0], F1[pos_ind1]
    subF0, subF1 = F0[sel0], F1[sel1]

    def pd(A, B):
        d2 = ((A * A).sum(1)[:, None] + (B * B).sum(1)[None, :]
              - 2.0 * (A @ B.T))
        return np.sqrt(np.maximum(d2, 0.0) + 1e-7)

    D01 = pd(posF0, subF1)
    D10 = pd(posF1, subF0)
    D01min, D10min = D01.min(1), D10.min(1)
    D01ind = np.asarray(sel1)[np.argmin(D01, 1)].astype(np.int64)
    D10ind = np.asarray(sel0)[np.argmin(D10, 1)].astype(np.int64)
    pos_keys = pos_ind0 + pos_ind1 * hash_seed
    mask0 = ~np.isin(pos_ind0 + D01ind * hash_seed, pos_keys)
    mask1 = ~np.isin(D10ind + pos_ind1 * hash_seed, pos_keys)
    pos_loss = np.mean(np.maximum(((posF0 - posF1) ** 2).sum(1) - POS_THRESH, 0))
    n0 = np.maximum(NEG_THRESH - D01min, 0) ** 2
    n1 = np.maximum(NEG_THRESH - D10min, 0) ** 2
    neg0 = (n0 * mask0).sum() / max(mask0.sum(), 1)
    neg1 = (n1 * mask1).sum() / max(mask1.sum(), 1)
    return np.float32(pos_loss + (neg0 + neg1) / 2.0)


def kernel(F0, F1, matches, sel0, sel1):
    global _CACHED_NC, LAST_RESULTS
    F0 = np.ascontiguousarray(np.asarray(F0), dtype=np.float32)
    F1 = np.ascontiguousarray(np.asarray(F1), dtype=np.float32)
    matches = np.asarray(matches)
    sel0 = np.asarray(sel0)
    sel1 = np.asarray(sel1)
    assert F0.shape == (N_PTS, D) and matches.shape == (P, 2)
    assert sel0.shape == (M,) and sel1.shape == (M,)

    in_maps = _prep_inputs(F0, F1, matches, sel0, sel1)
    if _CACHED_NC is None:
        _CACHED_NC = _build_nc()
    try:
        res = run_bass_kernel_spmd(_CACHED_NC, in_maps, list(range(N_CORES)))
    except Exception:
        # a wedged NeuronCore (e.g. NRT_EXEC_UNIT_UNRECOVERABLE from an
        # earlier crashed session) is recoverable via the axon reset call
        try:
            import ctypes

            lib = ctypes.CDLL("/opt/axon/libaxon_pjrt.so")
            lib.axon_reset.restype = ctypes.c_int64
            lib.axon_reset()
        except Exception:
            pass
        res = run_bass_kernel_spmd(_CACHED_NC, in_maps, list(range(N_CORES)))
    LAST_RESULTS = res
    outs = np.stack([r["out"] for r in res.results])   # (8, 1, 3)
    pos_sum = float(outs[:, 0, 0].sum())
    cert = float(outs[:, 0, 1].sum())
    if cert != 0.0:
        # some distance crossed the certificate threshold: the hardest
        # negatives (and the pair-mask) may now matter; recompute exactly.
        return _exact_host_reference(F0, F1, matches, sel0, sel1)
    return np.float32(pos_sum / P)
